# revision 15
# baseline (speedup 1.0000x reference)
"""Cross-attention + output projection + residual + GroupNorm on 8 NeuronCores.

Problem (hardcoded): B=4, C=256, H=W=48 (N=2304 pixels), 4 heads x 64 dim,
GroupNorm with 32 groups of 8 channels, eps=1e-5.

Sharding: 2 cores per batch element; each core handles one half of the
query pixels (1152) for all 4 heads.  K/V are computed for the full pixel
range on both cores of a pair (duplicated, cheap).  The only cross-core
communication is a 2KB AllReduce of per-channel (sum, sumsq) GroupNorm
partial statistics between the two cores of each pair.

v2 layout (all activations bf16, channel-major, pixels on the free axis):
  q = WqT.T @ xh + bq                  [256, 1152]
  k = WkT.T @ ctx + bk                 [256, 2304]
  vT[j, c] = (ctx.T @ WvT)[j, c]       [2304, 256] + ones column per head
  per head h (bf16 matmuls, j-tile jt of 128 keys, i-chunks of <=512):
    sT[j, i] = sum_d k_h[d, j] q_h[d, i]        (PE, K=64, i-chunked)
    eT = exp(0.125 * sT) -> bf16                (ACT)
    av[d+1, i] += vT_aug_h[j, :].T @ eT[j, i]   (PE: vT stationary [128,65],
                                                 streams 512 e-columns,
                                                 accumulated over 18 j-tiles;
                                                 row 64 = softmax denominator)
  normalize: r[1,i] = 1/av[64,i]; rbc[64,i] = ones.T @ r (PE k=1 broadcast);
             ao = av[0:64] * rbc (DVE) -> channel-major bf16, no transpose.
  y = WoT.T @ ao + (bo + Wo bv) + xh   [256, 1152]
  per-channel (sum, sumsq) via fused DVE accum -> pair AllReduce ->
  group stats via 0/1 selection matmuls -> y = y * a + b.
"""

import sys

if "/opt/trn_rl_repo" not in sys.path:
    sys.path.insert(0, "/opt/trn_rl_repo")

import numpy as np
import ml_dtypes

import concourse.bass as bass
import concourse.mybir as mybir
import concourse.tile as tile
from concourse import bacc
from concourse.bass_utils import run_bass_kernel_spmd

F32 = mybir.dt.float32
BF16 = mybir.dt.bfloat16
AF = mybir.ActivationFunctionType
ALU = mybir.AluOpType
NPBF16 = ml_dtypes.bfloat16

B, C, HW = 4, 256, 2304
NH, HD = 4, 64
NHALF = HW // 2  # 1152 query pixels per core
SCALE = HD ** -0.5  # 0.125
GSIZE = 8  # channels per GroupNorm group
EPS = 1e-5
GN_COUNT = GSIZE * HW  # elements per group per batch (after pair AllReduce)
NJT = HW // 128  # 18 key tiles of 128
SLICES = [(0, 512), (512, 1024), (1024, NHALF)]  # i-chunks (psum-bank sized)

_CACHE = {}


def _finalize(nc):
    """compile() leaves 3+-wait Matmults that walrus rejects ("Too many sync
    wait commands" on the S3_LW struct); a second compile pass — run here via
    finalize() — splits them onto EventSemaphores.  Verify that it worked."""
    nc.compile()
    nc.finalize()
    for fn in nc.m.functions:
        for bb in fn.blocks:
            for inst in bb.instructions:
                si = inst.sync_info
                if isinstance(inst, mybir.InstMatmult) and si is not None:
                    assert len(si.on_wait or []) <= 2, (inst.name, si.on_wait)


def _build():
    nc = bacc.Bacc("TRN2", target_bir_lowering=False, debug=False, num_devices=8)

    xh_d = nc.dram_tensor("xh", [C, NHALF], BF16, kind="ExternalInput").ap()
    ctx_d = nc.dram_tensor("ctx", [C, HW], BF16, kind="ExternalInput").ap()
    # packed weights: [C_in, (wq|wk|wv|wo) x C_out]
    wall_d = nc.dram_tensor("wall", [C, 4 * C], BF16, kind="ExternalInput").ap()
    # packed per-channel vectors: (bo_eff, gamma, beta, bq, bk, pad)
    ball_d = nc.dram_tensor("ball", [C, 6], F32, kind="ExternalInput").ap()
    gsel_d = nc.dram_tensor("gsel", [128, 16], F32, kind="ExternalInput").ap()
    gselT_d = nc.dram_tensor("gselT", [16, 128], F32, kind="ExternalInput").ap()
    yh_d = nc.dram_tensor("yh", [C, NHALF], F32, kind="ExternalOutput").ap()

    WQ, WK, WV, WO = 0, C, 2 * C, 3 * C  # column offsets in wall
    BO, GAMMA, BETA, BQ, BK = 0, 1, 2, 3, 4  # columns of ball

    with tile.TileContext(nc) as tc:
        with (
            tc.tile_pool(name="const", bufs=1) as const,
            tc.tile_pool(name="main", bufs=1) as main,
            tc.tile_pool(name="small", bufs=4) as small,
            tc.tile_pool(name="dram", bufs=2, space="DRAM") as dram,
        ):
            # ---- input DMAs (issue order == arrival order) ----
            wall_sb = const.tile([128, 2, 4 * C], BF16, tag="wall")
            wall_r = wall_d.rearrange("(k p) o -> p k o", p=128)
            for k in range(2):
                nc.sync.dma_start(out=wall_sb[:, k, :], in_=wall_r[:, k, :])
            x_sb = main.tile([128, 2, NHALF], BF16, tag="x")
            x_r = xh_d.rearrange("(k p) i -> p k i", p=128)
            for k in range(2):
                for s, e in SLICES:
                    nc.sync.dma_start(out=x_sb[:, k, s:e], in_=x_r[:, k, s:e])
            ctx_sb = main.tile([128, 2, HW], BF16, tag="ctx")
            ctx_r = ctx_d.rearrange("(k p) j -> p k j", p=128)
            for k in range(2):
                for jh in range(2):
                    o = jh * NHALF
                    for s, e in SLICES:
                        nc.sync.dma_start(
                            out=ctx_sb[:, k, o + s : o + e],
                            in_=ctx_r[:, k, o + s : o + e],
                        )
            ball_sb = const.tile([128, 2, 6], F32, tag="ball")
            nc.sync.dma_start(
                out=ball_sb, in_=ball_d.rearrange("(k p) s -> p k s", p=128)
            )
            gsel_sb = const.tile([128, 16], F32, tag="gsel")
            nc.sync.dma_start(out=gsel_sb, in_=gsel_d)
            gselT_sb = const.tile([16, 128], F32, tag="gselT")
            nc.sync.dma_start(out=gselT_sb, in_=gselT_d)

            eps_sb = const.tile([16, 1], F32, tag="eps")
            nc.vector.memset(eps_sb, EPS)

            # ---- long-lived activations ----
            q_sb = main.tile([128, 2, NHALF], BF16, tag="q")
            k_sb = main.tile([128, 2, HW], BF16, tag="k")
            # per head: [v columns (64) | ones columns (64)] -> the AV matmul
            # replicates the softmax denominator into psum partitions 64..127
            # at zero extra cost (PE time scales with moving columns only)
            vT_sb = main.tile([128, NJT, NH * 2 * HD], BF16, tag="vT")
            ao_sb = main.tile([128, 2, NHALF], BF16, tag="ao")
            y_sb = main.tile([128, 2, NHALF], F32, tag="y")
            scr = main.tile([128, NHALF], F32, tag="scr")
            stats_sb = small.tile([128, 2, 2], F32, tag="stats", bufs=1)

            # ones columns of vT (64 per head, strided over j-tiles)
            for h in range(NH):
                c0 = h * 2 * HD + HD
                nc.vector.memset(vT_sb[:, :, c0 : c0 + HD], 1.0)

            # ---- projections (bf16 matmuls, psum f32) ----
            with (
                tc.tile_pool(name="pp", bufs=2, space="PSUM") as pp,
                tc.tile_pool(name="vp", bufs=2, space="PSUM") as vp,
            ):
                # Q: [o_grp 128, 1152] -> q_sb bf16
                for g in range(2):
                    ps = pp.tile([128, NHALF], F32, tag="qk")
                    for k in range(2):
                        lhsT = wall_sb[:, k, WQ + g * 128 : WQ + (g + 1) * 128]
                        for s, e in SLICES:
                            nc.tensor.matmul(
                                ps[:, s:e], lhsT, x_sb[:, k, s:e],
                                start=(k == 0), stop=(k == 1),
                            )
                    nc.vector.tensor_scalar_add(
                        out=q_sb[:, g, :], in0=ps,
                        scalar1=ball_sb[:, g, BQ : BQ + 1],
                    )
                    if g == 0:
                        # K(g0): needed by heads 0/1 first
                        for jh in range(2):
                            ps = pp.tile([128, NHALF], F32, tag="qk")
                            for k in range(2):
                                lhsT = wall_sb[:, k, WK : WK + 128]
                                for s, e in SLICES:
                                    nc.tensor.matmul(
                                        ps[:, s:e], lhsT,
                                        ctx_sb[:, k, jh * NHALF + s : jh * NHALF + e],
                                        start=(k == 0), stop=(k == 1),
                                    )
                            nc.vector.tensor_scalar_add(
                                out=k_sb[:, 0, jh * NHALF : (jh + 1) * NHALF],
                                in0=ps, scalar1=ball_sb[:, 0, BK : BK + 1],
                            )
                        # V transposed: [j_tile 128, 256] -> vT_sb bf16
                        for jt in range(NJT):
                            ps = vp.tile([128, C], F32, tag="vp")
                            for k in range(2):
                                nc.tensor.matmul(
                                    ps, ctx_sb[:, k, jt * 128 : (jt + 1) * 128],
                                    wall_sb[:, k, WV : WV + C],
                                    start=(k == 0), stop=(k == 1),
                                )
                            nc.vector.tensor_copy(
                                out=vT_sb[:, jt, :]
                                .rearrange("p (h c) -> p h c", h=NH)[:, :, 0:HD],
                                in_=ps.rearrange("p (h c) -> p h c", h=NH),
                            )
                        # K(g1)
                        for jh in range(2):
                            ps = pp.tile([128, NHALF], F32, tag="qk")
                            for k in range(2):
                                lhsT = wall_sb[:, k, WK + 128 : WK + 256]
                                for s, e in SLICES:
                                    nc.tensor.matmul(
                                        ps[:, s:e], lhsT,
                                        ctx_sb[:, k, jh * NHALF + s : jh * NHALF + e],
                                        start=(k == 0), stop=(k == 1),
                                    )
                            nc.vector.tensor_scalar_add(
                                out=k_sb[:, 1, jh * NHALF : (jh + 1) * NHALF],
                                in0=ps, scalar1=ball_sb[:, 1, BK : BK + 1],
                            )

            # ---- attention (bf16 matmuls, V-stationary AV) ----
            with (
                tc.tile_pool(name="ep", bufs=3) as ep,
                tc.tile_pool(name="rp", bufs=2) as rp,
                tc.tile_pool(name="scp", bufs=1, space="PSUM") as scp,
                tc.tile_pool(name="avp", bufs=1, space="PSUM") as avp,
            ):
                for h in range(NH):
                    g, off = h // 2, (h % 2) * HD
                    vcol = h * 2 * HD
                    av_t = [
                        avp.tile([2 * HD, e - s], F32, tag=f"av{ci}",
                                 name=f"av{ci}")
                        for ci, (s, e) in enumerate(SLICES)
                    ]
                    for jt in range(NJT):
                        lhsT = k_sb[off : off + HD, g, jt * 128 : (jt + 1) * 128]
                        sc_t = []
                        for ci, (s, e) in enumerate(SLICES):
                            ps = scp.tile([128, e - s], F32, tag=f"s{ci}")
                            nc.tensor.matmul(
                                ps, lhsT, q_sb[off : off + HD, g, s:e],
                                start=True, stop=True,
                            )
                            sc_t.append(ps)
                        ex = ep.tile([128, NHALF], BF16, tag="e")
                        for ci, (s, e) in enumerate(SLICES):
                            nc.scalar.activation(
                                out=ex[:, s:e], in_=sc_t[ci],
                                func=AF.Exp, scale=SCALE,
                            )
                        for ci, (s, e) in enumerate(SLICES):
                            nc.tensor.matmul(
                                av_t[ci],
                                vT_sb[:, jt, vcol : vcol + 2 * HD],
                                ex[:, s:e],
                                start=(jt == 0), stop=(jt == NJT - 1),
                            )
                    # normalize: ao = av[0:64] / denom; the denominator sits
                    # replicated in av partitions 64..127
                    rb = rp.tile([HD, NHALF], BF16, tag="rb")
                    with nc.allow_low_precision(
                        reason="softmax denom reciprocal stored bf16; "
                        "~0.4% rel, gate is 2e-2"
                    ):
                        for ci, (s, e) in enumerate(SLICES):
                            nc.vector.reciprocal(
                                out=rb[:, s:e], in_=av_t[ci][HD : 2 * HD, :]
                            )
                    for ci, (s, e) in enumerate(SLICES):
                        nc.vector.tensor_mul(
                            out=ao_sb[off : off + HD, g, s:e],
                            in0=av_t[ci][0:HD, :], in1=rb[:, s:e],
                        )

            # ---- output projection + residual + GroupNorm stats ----
            with (
                tc.tile_pool(name="wop", bufs=2, space="PSUM") as wop,
                tc.tile_pool(name="gnp", bufs=1, space="PSUM") as gnp,
            ):
                for g in range(2):
                    ps = wop.tile([128, NHALF], F32, tag="wo")
                    for k in range(2):
                        lhsT = wall_sb[:, k, WO + g * 128 : WO + (g + 1) * 128]
                        for s, e in SLICES:
                            nc.tensor.matmul(
                                ps[:, s:e], lhsT, ao_sb[:, k, s:e],
                                start=(k == 0), stop=(k == 1),
                            )
                    # y = ps + bo_eff + x ; accum -> per-channel sum
                    nc.vector.scalar_tensor_tensor(
                        out=y_sb[:, g, :], in0=ps,
                        scalar=ball_sb[:, g, BO : BO + 1],
                        in1=x_sb[:, g, :], op0=ALU.add, op1=ALU.add,
                        accum_out=stats_sb[:, g, 0:1],
                    )
                    # sumsq (tensor_tensor_reduce crashes the DVE exec unit
                    # on this hw, so square+reduce in two passes)
                    nc.vector.tensor_mul(
                        out=scr, in0=y_sb[:, g, :], in1=y_sb[:, g, :]
                    )
                    nc.vector.reduce_sum(
                        out=stats_sb[:, g, 1:2], in_=scr,
                        axis=mybir.AxisListType.X,
                    )

                # pair AllReduce of per-channel (sum, sumsq)
                gn_in = dram.tile([C, 2], F32, tag="gnin", bufs=1)
                gn_out = dram.tile([C, 2], F32, tag="gnout", bufs=1)
                nc.sync.dma_start(
                    out=gn_in.rearrange("(k p) s -> p k s", p=128), in_=stats_sb
                )
                nc.gpsimd.collective_compute(
                    "AllReduce", ALU.add,
                    replica_groups=[[0, 1], [2, 3], [4, 5], [6, 7]],
                    ins=[gn_in.opt()], outs=[gn_out.opt()],
                )
                gs_sb = small.tile([128, 2, 2], F32, tag="gs", bufs=1)
                nc.sync.dma_start(
                    out=gs_sb, in_=gn_out.rearrange("(k p) s -> p k s", p=128)
                )

                # group totals via 0/1 selection matmul: [16, (k, sum|sumsq)]
                gp = gnp.tile([16, 4], F32, tag="gp")
                nc.tensor.matmul(gp, gsel_sb, gs_sb, start=True, stop=True)
                gpv = gp.rearrange("p (k s) -> p k s", k=2)
                m16 = small.tile([16, 2], F32, tag="m16", bufs=1)
                v16 = small.tile([16, 2], F32, tag="v16", bufs=1)
                st16 = small.tile([16, 2, 2], F32, tag="st16", bufs=1)
                nc.scalar.mul(out=m16, in_=gpv[:, :, 0], mul=1.0 / GN_COUNT)
                nc.scalar.mul(out=v16, in_=gpv[:, :, 1], mul=1.0 / GN_COUNT)
                nc.vector.tensor_copy(out=st16[:, :, 0], in_=m16)
                nc.vector.tensor_mul(out=m16, in0=m16, in1=m16)
                nc.vector.tensor_tensor(
                    out=v16, in0=v16, in1=m16, op=ALU.subtract
                )
                nc.scalar.activation(out=v16, in_=v16, func=AF.Sqrt, bias=eps_sb)
                nc.vector.reciprocal(out=st16[:, :, 1], in_=v16)

                # broadcast (mean, rstd) to channels, fold gamma/beta:
                # y_out = y * a + b,  a = rstd*gamma,  b = beta - mean*a
                bc = gnp.tile([128, 4], F32, tag="bc")
                nc.tensor.matmul(bc, gselT_sb, st16, start=True, stop=True)
                bcv = bc.rearrange("p (k s) -> p k s", k=2)
                ab = small.tile([128, 2, 2], F32, tag="ab", bufs=1)
                tmp = small.tile([128, 2], F32, tag="tmp", bufs=1)
                nc.vector.tensor_mul(
                    out=ab[:, :, 0], in0=bcv[:, :, 1], in1=ball_sb[:, :, GAMMA]
                )
                nc.vector.tensor_mul(out=tmp, in0=bcv[:, :, 0], in1=ab[:, :, 0])
                nc.vector.tensor_tensor(
                    out=ab[:, :, 1], in0=ball_sb[:, :, BETA], in1=tmp,
                    op=ALU.subtract,
                )

                yr = yh_d.rearrange("(k p) i -> p k i", p=128)
                for g in range(2):
                    for s, e in ((0, 576), (576, NHALF)):
                        nc.vector.tensor_scalar(
                            out=y_sb[:, g, s:e], in0=y_sb[:, g, s:e],
                            scalar1=ab[:, g, 0:1], scalar2=ab[:, g, 1:2],
                            op0=ALU.mult, op1=ALU.add,
                        )
                        nc.sync.dma_start(out=yr[:, g, s:e], in_=y_sb[:, g, s:e])

    _finalize(nc)
    return nc


def _get_nc():
    if "nc" not in _CACHE:
        _CACHE["nc"] = _build()
    return _CACHE["nc"]


def make_in_maps(x, context, Wq, bq, Wk, bk, Wv, bv, Wo, bo, gamma, beta):
    x = np.asarray(x, np.float32)
    context = np.asarray(context, np.float32)
    xr = np.ascontiguousarray(x.reshape(B, C, HW)).astype(NPBF16)
    cr = np.ascontiguousarray(context.reshape(B, C, HW)).astype(NPBF16)

    Wq = np.asarray(Wq, np.float32)
    Wk = np.asarray(Wk, np.float32)
    Wv = np.asarray(Wv, np.float32)
    Wo = np.asarray(Wo, np.float32)
    bo_eff = np.asarray(bo, np.float32) + Wo @ np.asarray(bv, np.float32)

    wall = np.concatenate([Wq.T, Wk.T, Wv.T, Wo.T], axis=1).astype(NPBF16)
    ball = np.stack(
        [
            bo_eff,
            np.asarray(gamma, np.float32),
            np.asarray(beta, np.float32),
            np.asarray(bq, np.float32),
            np.asarray(bk, np.float32),
            np.zeros(C, np.float32),
        ],
        axis=1,
    )

    gsel = np.zeros((128, 16), np.float32)
    gsel[np.arange(128), np.arange(128) // GSIZE] = 1.0

    shared = {
        "wall": np.ascontiguousarray(wall),
        "ball": np.ascontiguousarray(ball),
        "gsel": gsel,
        "gselT": np.ascontiguousarray(gsel.T),
    }
    in_maps = []
    for core in range(8):
        b, half = core // 2, core % 2
        m = dict(shared)
        m["xh"] = np.ascontiguousarray(xr[b, :, half * NHALF : (half + 1) * NHALF])
        m["ctx"] = cr[b]
        in_maps.append(m)
    return in_maps


def kernel(x, context, Wq, bq, Wk, bk, Wv, bv, Wo, bo, gamma, beta):
    in_maps = make_in_maps(
        x, context, Wq, bq, Wk, bk, Wv, bv, Wo, bo, gamma, beta
    )
    x = np.asarray(x, np.float32)

    nc = _get_nc()
    res = run_bass_kernel_spmd(nc, in_maps, core_ids=list(range(8)))

    out = np.empty((B, C, HW), np.float32)
    for core in range(8):
        b, half = core // 2, core % 2
        out[b, :, half * NHALF : (half + 1) * NHALF] = res.results[core]["yh"]
    return out.reshape(x.shape)


# revision 17
# speedup vs baseline: 1.3289x; 1.3289x over previous
"""Cross-attention + output projection + residual + GroupNorm on 8 NeuronCores.

Problem (hardcoded): B=4, C=256, H=W=48 (N=2304 pixels), 4 heads x 64 dim,
GroupNorm with 32 groups of 8 channels, eps=1e-5.

Sharding: 2 cores per batch element; each core handles one half of the
query pixels (1152) for all 4 heads.  K/V are computed for the full pixel
range on both cores of a pair (duplicated, cheap).  The only cross-core
communication is a 2KB AllReduce of per-channel (sum, sumsq) GroupNorm
partial statistics between the two cores of each pair.

v2 layout (all activations bf16, channel-major, pixels on the free axis):
  q = WqT.T @ xh + bq                  [256, 1152]
  k = WkT.T @ ctx + bk                 [256, 2304]
  vT[j, c] = (ctx.T @ WvT)[j, c]       [2304, 256] + ones column per head
  per head h (bf16 matmuls, j-tile jt of 128 keys, i-chunks of <=512):
    sT[j, i] = sum_d k_h[d, j] q_h[d, i]        (PE, K=64, i-chunked)
    eT = exp(0.125 * sT) -> bf16                (ACT)
    av[d+1, i] += vT_aug_h[j, :].T @ eT[j, i]   (PE: vT stationary [128,65],
                                                 streams 512 e-columns,
                                                 accumulated over 18 j-tiles;
                                                 row 64 = softmax denominator)
  normalize: r[1,i] = 1/av[64,i]; rbc[64,i] = ones.T @ r (PE k=1 broadcast);
             ao = av[0:64] * rbc (DVE) -> channel-major bf16, no transpose.
  y = WoT.T @ ao + (bo + Wo bv) + xh   [256, 1152]
  per-channel (sum, sumsq) via fused DVE accum -> pair AllReduce ->
  group stats via 0/1 selection matmuls -> y = y * a + b.
"""

import sys

if "/opt/trn_rl_repo" not in sys.path:
    sys.path.insert(0, "/opt/trn_rl_repo")

import numpy as np
import ml_dtypes

import concourse.bass as bass
import concourse.mybir as mybir
import concourse.tile as tile
from concourse import bacc
from concourse.bass_utils import run_bass_kernel_spmd

F32 = mybir.dt.float32
BF16 = mybir.dt.bfloat16
AF = mybir.ActivationFunctionType
ALU = mybir.AluOpType
NPBF16 = ml_dtypes.bfloat16

B, C, HW = 4, 256, 2304
NH, HD = 4, 64
NHALF = HW // 2  # 1152 query pixels per core
SCALE = HD ** -0.5  # 0.125
GSIZE = 8  # channels per GroupNorm group
EPS = 1e-5
GN_COUNT = GSIZE * HW  # elements per group per batch (after pair AllReduce)
NJT = HW // 128  # 18 key tiles of 128
SLICES = [(0, 512), (512, 1024), (1024, NHALF)]  # i-chunks (psum-bank sized)

_CACHE = {}


def _finalize(nc):
    """compile() leaves 3+-wait Matmults that walrus rejects ("Too many sync
    wait commands" on the S3_LW struct); a second compile pass — run here via
    finalize() — splits them onto EventSemaphores.  Verify that it worked."""
    nc.compile()
    nc.finalize()
    for fn in nc.m.functions:
        for bb in fn.blocks:
            for inst in bb.instructions:
                si = inst.sync_info
                if isinstance(inst, mybir.InstMatmult) and si is not None:
                    assert len(si.on_wait or []) <= 2, (inst.name, si.on_wait)


def _build():
    nc = bacc.Bacc("TRN2", target_bir_lowering=False, debug=False, num_devices=8)

    xh_d = nc.dram_tensor("xh", [C, NHALF], BF16, kind="ExternalInput").ap()
    ctx_d = nc.dram_tensor("ctx", [C, HW], BF16, kind="ExternalInput").ap()
    # packed weights: [C_in, (wq|wk|wv|wo) x C_out]
    wall_d = nc.dram_tensor("wall", [C, 4 * C], BF16, kind="ExternalInput").ap()
    # packed per-channel vectors: (bo_eff, gamma, beta, bq, bk, pad)
    ball_d = nc.dram_tensor("ball", [C, 6], F32, kind="ExternalInput").ap()
    gsel_d = nc.dram_tensor("gsel", [128, 16], F32, kind="ExternalInput").ap()
    gselT_d = nc.dram_tensor("gselT", [16, 128], F32, kind="ExternalInput").ap()
    yh_d = nc.dram_tensor("yh", [C, NHALF], F32, kind="ExternalOutput").ap()

    WQ, WK, WV, WO = 0, C, 2 * C, 3 * C  # column offsets in wall
    BO, GAMMA, BETA, BQ, BK = 0, 1, 2, 3, 4  # columns of ball

    with tile.TileContext(nc) as tc:
        with (
            tc.tile_pool(name="const", bufs=1) as const,
            tc.tile_pool(name="main", bufs=1) as main,
            tc.tile_pool(name="small", bufs=4) as small,
            tc.tile_pool(name="dram", bufs=2, space="DRAM") as dram,
        ):
            # ---- input DMAs (issue order == arrival order) ----
            wall_sb = const.tile([128, 2, 4 * C], BF16, tag="wall")
            wall_r = wall_d.rearrange("(k p) o -> p k o", p=128)
            for k in range(2):
                nc.sync.dma_start(out=wall_sb[:, k, :], in_=wall_r[:, k, :])
            x_sb = main.tile([128, 2, NHALF], BF16, tag="x")
            x_r = xh_d.rearrange("(k p) i -> p k i", p=128)
            for k in range(2):
                for s, e in SLICES:
                    nc.sync.dma_start(out=x_sb[:, k, s:e], in_=x_r[:, k, s:e])
            ctx_sb = main.tile([128, 2, HW], BF16, tag="ctx")
            ctx_r = ctx_d.rearrange("(k p) j -> p k j", p=128)
            for k in range(2):
                for jh in range(2):
                    o = jh * NHALF
                    for s, e in SLICES:
                        nc.sync.dma_start(
                            out=ctx_sb[:, k, o + s : o + e],
                            in_=ctx_r[:, k, o + s : o + e],
                        )
            ball_sb = const.tile([128, 2, 6], F32, tag="ball")
            nc.sync.dma_start(
                out=ball_sb, in_=ball_d.rearrange("(k p) s -> p k s", p=128)
            )
            gsel_sb = const.tile([128, 16], F32, tag="gsel")
            nc.sync.dma_start(out=gsel_sb, in_=gsel_d)
            gselT_sb = const.tile([16, 128], F32, tag="gselT")
            nc.sync.dma_start(out=gselT_sb, in_=gselT_d)

            eps_sb = const.tile([16, 1], F32, tag="eps")
            nc.vector.memset(eps_sb, EPS)

            # ---- long-lived activations ----
            q_sb = main.tile([128, 2, NHALF], BF16, tag="q")
            k_sb = main.tile([128, 2, HW], BF16, tag="k")
            # per head: [v columns (64) | ones columns (64)] -> the AV matmul
            # replicates the softmax denominator into psum partitions 64..127
            # at zero extra cost (PE time scales with moving columns only)
            vT_sb = main.tile([128, NJT, NH * 2 * HD], BF16, tag="vT")
            ao_sb = main.tile([128, 2, NHALF], BF16, tag="ao")
            y_sb = main.tile([128, 2, NHALF], F32, tag="y")
            scr = main.tile([128, NHALF], F32, tag="scr")
            stats_sb = small.tile([128, 2, 2], F32, tag="stats", bufs=1)

            # ones columns of vT (64 per head, strided over j-tiles)
            for h in range(NH):
                c0 = h * 2 * HD + HD
                nc.vector.memset(vT_sb[:, :, c0 : c0 + HD], 1.0)

            # ---- projections (bf16 matmuls, psum f32) ----
            with (
                tc.tile_pool(name="pp", bufs=2, space="PSUM") as pp,
                tc.tile_pool(name="vp", bufs=2, space="PSUM") as vp,
            ):
                # Q: [o_grp 128, 1152] -> q_sb bf16
                for g in range(2):
                    ps = pp.tile([128, NHALF], F32, tag="qk")
                    for k in range(2):
                        lhsT = wall_sb[:, k, WQ + g * 128 : WQ + (g + 1) * 128]
                        for s, e in SLICES:
                            nc.tensor.matmul(
                                ps[:, s:e], lhsT, x_sb[:, k, s:e],
                                start=(k == 0), stop=(k == 1),
                            )
                    nc.vector.tensor_scalar_add(
                        out=q_sb[:, g, :], in0=ps,
                        scalar1=ball_sb[:, g, BQ : BQ + 1],
                    )
                    if g == 0:
                        # K(g0): needed by heads 0/1 first
                        for jh in range(2):
                            ps = pp.tile([128, NHALF], F32, tag="qk")
                            for k in range(2):
                                lhsT = wall_sb[:, k, WK : WK + 128]
                                for s, e in SLICES:
                                    nc.tensor.matmul(
                                        ps[:, s:e], lhsT,
                                        ctx_sb[:, k, jh * NHALF + s : jh * NHALF + e],
                                        start=(k == 0), stop=(k == 1),
                                    )
                            nc.vector.tensor_scalar_add(
                                out=k_sb[:, 0, jh * NHALF : (jh + 1) * NHALF],
                                in0=ps, scalar1=ball_sb[:, 0, BK : BK + 1],
                            )
                        # V transposed: [j_tile 128, 256] -> vT_sb bf16
                        for jt in range(NJT):
                            ps = vp.tile([128, C], F32, tag="vp")
                            for k in range(2):
                                nc.tensor.matmul(
                                    ps, ctx_sb[:, k, jt * 128 : (jt + 1) * 128],
                                    wall_sb[:, k, WV : WV + C],
                                    start=(k == 0), stop=(k == 1),
                                )
                            nc.vector.tensor_copy(
                                out=vT_sb[:, jt, :]
                                .rearrange("p (h c) -> p h c", h=NH)[:, :, 0:HD],
                                in_=ps.rearrange("p (h c) -> p h c", h=NH),
                            )
                        # K(g1)
                        for jh in range(2):
                            ps = pp.tile([128, NHALF], F32, tag="qk")
                            for k in range(2):
                                lhsT = wall_sb[:, k, WK + 128 : WK + 256]
                                for s, e in SLICES:
                                    nc.tensor.matmul(
                                        ps[:, s:e], lhsT,
                                        ctx_sb[:, k, jh * NHALF + s : jh * NHALF + e],
                                        start=(k == 0), stop=(k == 1),
                                    )
                            nc.vector.tensor_scalar_add(
                                out=k_sb[:, 1, jh * NHALF : (jh + 1) * NHALF],
                                in0=ps, scalar1=ball_sb[:, 1, BK : BK + 1],
                            )

            # ---- attention (bf16 matmuls, V-stationary AV) ----
            # Software-pipelined: the AV matmuls of j-tile jt run between the
            # scores of jt+1 and jt+2, so the PE never waits on the ACT exp
            # and stays out of the slow p-state.  PSUM: s0/s1 double-buffered
            # (4 banks) + s2 (1) + one 1152-wide AV accumulator (3) = 8.
            with (
                tc.tile_pool(name="ep", bufs=4) as ep,
                tc.tile_pool(name="rp", bufs=2) as rp,
                tc.tile_pool(name="scp", bufs=1, space="PSUM") as scp,
                tc.tile_pool(name="avp", bufs=1, space="PSUM") as avp,
            ):
                for h in range(NH):
                    g, off = h // 2, (h % 2) * HD
                    vcol = h * 2 * HD
                    av = avp.tile([128, NHALF], F32, tag="av")
                    ex_t = [None] * NJT

                    def emit_scores_exp(jt):
                        lhsT = k_sb[off : off + HD, g, jt * 128 : (jt + 1) * 128]
                        sc_t = []
                        for ci, (s, e) in enumerate(SLICES):
                            ps = scp.tile(
                                [128, e - s], F32, tag=f"s{ci}",
                                name=f"s{ci}", bufs=(2 if ci < 2 else 1),
                            )
                            nc.tensor.matmul(
                                ps, lhsT, q_sb[off : off + HD, g, s:e],
                                start=True, stop=True,
                            )
                            sc_t.append(ps)
                        ex = ep.tile([128, NHALF], BF16, tag="e")
                        for ci, (s, e) in enumerate(SLICES):
                            nc.scalar.activation(
                                out=ex[:, s:e], in_=sc_t[ci],
                                func=AF.Exp, scale=SCALE,
                            )
                        ex_t[jt] = ex

                    def emit_av(jt):
                        for ci, (s, e) in enumerate(SLICES):
                            nc.tensor.matmul(
                                av[:, s:e],
                                vT_sb[:, jt, vcol : vcol + 2 * HD],
                                ex_t[jt][:, s:e],
                                start=(jt == 0), stop=(jt == NJT - 1),
                            )
                        ex_t[jt] = None

                    for jt in range(NJT):
                        emit_scores_exp(jt)
                        if jt > 0:
                            emit_av(jt - 1)
                    emit_av(NJT - 1)

                    # normalize: ao = av[0:64] / denom; the denominator sits
                    # replicated in av partitions 64..127
                    rb = rp.tile([HD, NHALF], BF16, tag="rb")
                    with nc.allow_low_precision(
                        reason="softmax denom reciprocal stored bf16; "
                        "~0.4% rel, gate is 2e-2"
                    ):
                        nc.vector.reciprocal(out=rb, in_=av[HD:128, :])
                    nc.vector.tensor_mul(
                        out=ao_sb[off : off + HD, g, :],
                        in0=av[0:HD, :], in1=rb,
                    )

            # ---- output projection + residual + GroupNorm stats ----
            with (
                tc.tile_pool(name="wop", bufs=2, space="PSUM") as wop,
                tc.tile_pool(name="gnp", bufs=1, space="PSUM") as gnp,
            ):
                for g in range(2):
                    ps = wop.tile([128, NHALF], F32, tag="wo")
                    for k in range(2):
                        lhsT = wall_sb[:, k, WO + g * 128 : WO + (g + 1) * 128]
                        for s, e in SLICES:
                            nc.tensor.matmul(
                                ps[:, s:e], lhsT, ao_sb[:, k, s:e],
                                start=(k == 0), stop=(k == 1),
                            )
                    # y = ps + bo_eff + x ; accum -> per-channel sum
                    nc.vector.scalar_tensor_tensor(
                        out=y_sb[:, g, :], in0=ps,
                        scalar=ball_sb[:, g, BO : BO + 1],
                        in1=x_sb[:, g, :], op0=ALU.add, op1=ALU.add,
                        accum_out=stats_sb[:, g, 0:1],
                    )
                    # sumsq on the (tail-idle) ACT engine, overlapping the
                    # DVE pass above (tensor_tensor_reduce crashes the DVE
                    # exec unit on this hw)
                    nc.scalar.activation(
                        out=scr, in_=y_sb[:, g, :], func=AF.Square,
                        accum_out=stats_sb[:, g, 1:2],
                    )

                # pair AllReduce of per-channel (sum, sumsq)
                gn_in = dram.tile([C, 2], F32, tag="gnin", bufs=1)
                gn_out = dram.tile([C, 2], F32, tag="gnout", bufs=1)
                nc.sync.dma_start(
                    out=gn_in.rearrange("(k p) s -> p k s", p=128), in_=stats_sb
                )
                nc.gpsimd.collective_compute(
                    "AllReduce", ALU.add,
                    replica_groups=[[0, 1], [2, 3], [4, 5], [6, 7]],
                    ins=[gn_in.opt()], outs=[gn_out.opt()],
                )
                gs_sb = small.tile([128, 2, 2], F32, tag="gs", bufs=1)
                nc.sync.dma_start(
                    out=gs_sb, in_=gn_out.rearrange("(k p) s -> p k s", p=128)
                )

                # group totals via 0/1 selection matmul: [16, (k, sum|sumsq)]
                gp = gnp.tile([16, 4], F32, tag="gp")
                nc.tensor.matmul(gp, gsel_sb, gs_sb, start=True, stop=True)
                gpv = gp.rearrange("p (k s) -> p k s", k=2)
                m16 = small.tile([16, 2], F32, tag="m16", bufs=1)
                v16 = small.tile([16, 2], F32, tag="v16", bufs=1)
                st16 = small.tile([16, 2, 2], F32, tag="st16", bufs=1)
                nc.scalar.mul(out=m16, in_=gpv[:, :, 0], mul=1.0 / GN_COUNT)
                nc.scalar.mul(out=v16, in_=gpv[:, :, 1], mul=1.0 / GN_COUNT)
                nc.vector.tensor_copy(out=st16[:, :, 0], in_=m16)
                nc.vector.tensor_mul(out=m16, in0=m16, in1=m16)
                nc.vector.tensor_tensor(
                    out=v16, in0=v16, in1=m16, op=ALU.subtract
                )
                nc.scalar.activation(out=v16, in_=v16, func=AF.Sqrt, bias=eps_sb)
                nc.vector.reciprocal(out=st16[:, :, 1], in_=v16)

                # broadcast (mean, rstd) to channels, fold gamma/beta:
                # y_out = y * a + b,  a = rstd*gamma,  b = beta - mean*a
                bc = gnp.tile([128, 4], F32, tag="bc")
                nc.tensor.matmul(bc, gselT_sb, st16, start=True, stop=True)
                bcv = bc.rearrange("p (k s) -> p k s", k=2)
                ab = small.tile([128, 2, 2], F32, tag="ab", bufs=1)
                tmp = small.tile([128, 2], F32, tag="tmp", bufs=1)
                nc.vector.tensor_mul(
                    out=ab[:, :, 0], in0=bcv[:, :, 1], in1=ball_sb[:, :, GAMMA]
                )
                nc.vector.tensor_mul(out=tmp, in0=bcv[:, :, 0], in1=ab[:, :, 0])
                nc.vector.tensor_tensor(
                    out=ab[:, :, 1], in0=ball_sb[:, :, BETA], in1=tmp,
                    op=ALU.subtract,
                )

                yr = yh_d.rearrange("(k p) i -> p k i", p=128)
                for g in range(2):
                    for s, e in ((0, 576), (576, NHALF)):
                        nc.vector.tensor_scalar(
                            out=y_sb[:, g, s:e], in0=y_sb[:, g, s:e],
                            scalar1=ab[:, g, 0:1], scalar2=ab[:, g, 1:2],
                            op0=ALU.mult, op1=ALU.add,
                        )
                        nc.sync.dma_start(out=yr[:, g, s:e], in_=y_sb[:, g, s:e])

    _finalize(nc)
    return nc


def _get_nc():
    if "nc" not in _CACHE:
        _CACHE["nc"] = _build()
    return _CACHE["nc"]


def make_in_maps(x, context, Wq, bq, Wk, bk, Wv, bv, Wo, bo, gamma, beta):
    x = np.asarray(x, np.float32)
    context = np.asarray(context, np.float32)
    xr = np.ascontiguousarray(x.reshape(B, C, HW)).astype(NPBF16)
    cr = np.ascontiguousarray(context.reshape(B, C, HW)).astype(NPBF16)

    Wq = np.asarray(Wq, np.float32)
    Wk = np.asarray(Wk, np.float32)
    Wv = np.asarray(Wv, np.float32)
    Wo = np.asarray(Wo, np.float32)
    bo_eff = np.asarray(bo, np.float32) + Wo @ np.asarray(bv, np.float32)

    wall = np.concatenate([Wq.T, Wk.T, Wv.T, Wo.T], axis=1).astype(NPBF16)
    ball = np.stack(
        [
            bo_eff,
            np.asarray(gamma, np.float32),
            np.asarray(beta, np.float32),
            np.asarray(bq, np.float32),
            np.asarray(bk, np.float32),
            np.zeros(C, np.float32),
        ],
        axis=1,
    )

    gsel = np.zeros((128, 16), np.float32)
    gsel[np.arange(128), np.arange(128) // GSIZE] = 1.0

    shared = {
        "wall": np.ascontiguousarray(wall),
        "ball": np.ascontiguousarray(ball),
        "gsel": gsel,
        "gselT": np.ascontiguousarray(gsel.T),
    }
    in_maps = []
    for core in range(8):
        b, half = core // 2, core % 2
        m = dict(shared)
        m["xh"] = np.ascontiguousarray(xr[b, :, half * NHALF : (half + 1) * NHALF])
        m["ctx"] = cr[b]
        in_maps.append(m)
    return in_maps


def kernel(x, context, Wq, bq, Wk, bk, Wv, bv, Wo, bo, gamma, beta):
    in_maps = make_in_maps(
        x, context, Wq, bq, Wk, bk, Wv, bv, Wo, bo, gamma, beta
    )
    x = np.asarray(x, np.float32)

    nc = _get_nc()
    res = run_bass_kernel_spmd(nc, in_maps, core_ids=list(range(8)))

    out = np.empty((B, C, HW), np.float32)
    for core in range(8):
        b, half = core // 2, core % 2
        out[b, :, half * NHALF : (half + 1) * NHALF] = res.results[core]["yh"]
    return out.reshape(x.shape)
